# revision 19
# baseline (speedup 1.0000x reference)
"""Trainium2 Bass kernel for DynamicResidualStageWrapper (18-block MLP stage
with channel-gated anchor routing), data-parallel over batch across 8 cores.

Contract: kernel(**inputs) takes FULL unsharded inputs (as numpy arrays, keyed
as in reference.setup_inputs()) and returns the FULL output [32,14,14,512].

Per-core layout: activations live transposed as [C=512, tokens=784] split into
4 partition-tiles [128, 784] in bf16; tokens are (sample b, position hw).
Block weights [cin, cout] are the natural lhsT for out[cout, tok] = W.T @ X.
The whole block path runs in bf16 (fp32 PSUM accumulate): rel-err ~4e-3 vs
the 2e-2 gate, and bf16 halves HBM traffic, halves LDWEIGHTS (FWL), and
doubles DVE throughput. Each (block, ct) uses ONE two-bank PSUM tile
[128,1024] (chunk01 at cols 0:392, chunk23 at 512:904) so gelu is a single
fat ACT instruction per ct — ACT is the closest co-bottleneck to the PE.
Post-target blocks (12, 15) are emitted chunk-major so the PE consumes the
router corrections at 2-sample granularity while the DVE produces them.
Softmax's exp uses e^x = (1+tanh(x/2))/(1-tanh(x/2)) so the scalar engine
never switches activation-table sets away from gelu/tanh.
"""

import numpy as np

import concourse.bacc as bacc
import concourse.mybir as mybir
import concourse.tile as tile
from concourse.bass_utils import run_bass_kernel_spmd

# ---- problem constants (hardcoded per spec) ----
NUM_BLOCKS = 18
ANCHOR_IDX = (1, 4, 9)
TARGET_IDX = (11, 14, 17)
C = 512
HID = 128
A = 3
B, H, W = 32, 14, 14
N_CORES = 8
BL = B // N_CORES          # 4 samples per core
HW = H * W                 # 196 positions per sample
T = BL * HW                # 784 tokens per core
KT = C // 128              # 4 channel tiles
CH = T // 2                # 392 tokens per chunk (2 samples)
KB = KT * BL               # 16 (k, b) gate columns per anchor

F32 = mybir.dt.float32
BF16 = mybir.dt.bfloat16
GELU = mybir.ActivationFunctionType.Gelu_apprx_tanh
TANH = mybir.ActivationFunctionType.Tanh
_cached = {}


def build_program():
    """Build the per-core Bass/Tile program (same program on all 8 cores)."""
    nc = bacc.Bacc(trn_type="TRN2", target_bir_lowering=False, debug=False)

    xT = nc.dram_tensor("xT", [C, T], BF16, kind="ExternalInput").ap()
    wd = nc.dram_tensor("wd", [NUM_BLOCKS, C, C], BF16, kind="ExternalInput").ap()
    bias_cols = nc.dram_tensor("bias_cols", [128, NUM_BLOCKS * KT], F32,
                               kind="ExternalInput").ap()
    fc1w = nc.dram_tensor("fc1w", [128, A * KT * 128], BF16, kind="ExternalInput").ap()
    fc1b = nc.dram_tensor("fc1b", [128, A], F32, kind="ExternalInput").ap()
    fc2w = nc.dram_tensor("fc2w", [128, A * A * C], BF16, kind="ExternalInput").ap()
    fc2bias = nc.dram_tensor("fc2bias", [128, A * A * KT * BL], F32,
                             kind="ExternalInput").ap()
    gbc = nc.dram_tensor("gbc", [128, A], F32, kind="ExternalInput").ap()
    outT = nc.dram_tensor("outT", [C, T], BF16, kind="ExternalOutput").ap()

    anchor_of = {b: i for i, b in enumerate(ANCHOR_IDX)}
    target_of = {b: i for i, b in enumerate(TARGET_IDX)}
    post_target = tuple(i + 1 for i in TARGET_IDX if i + 1 < NUM_BLOCKS)

    with tile.TileContext(nc) as tc:
        with (
            tc.tile_pool(name="const", bufs=1) as cpool,
            tc.tile_pool(name="wpool", bufs=6) as wpool,
            tc.tile_pool(name="xpool", bufs=2) as xpool,
            tc.tile_pool(name="apool", bufs=1) as apool,
            tc.tile_pool(name="rpool", bufs=2) as rpool,
            tc.tile_pool(name="ppool", bufs=3, space="PSUM") as ppool,
        ):
            # ---- startup DMAs: block-0 weights as four separate k-tiles so
            # the first matmul is gated by one 128KB DMA, not all four;
            # x tiles split across the scalar/gpsimd queues, router constants
            # behind them. ACT table warm-up (gelu+tanh) hides under the DMAs.
            w0k = []
            for k in range(KT):
                wk = cpool.tile([128, C], BF16, name=f"w0k{k}")
                nc.sync.dma_start(wk[:], wd[0, k * 128:(k + 1) * 128, :])
                w0k.append(wk)
            X = []
            for k in range(KT):
                xt = xpool.tile([128, T], BF16, tag=f"x{k}", name=f"xin{k}")
                eng = nc.scalar if k % 2 == 0 else nc.gpsimd
                if k == 0:
                    # split x0 so the first matmul's chunk lands sooner
                    eng.dma_start(xt[:, 0:CH], xT[0:128, 0:CH])
                    eng.dma_start(xt[:, CH:T], xT[0:128, CH:T])
                else:
                    eng.dma_start(xt[:], xT[k * 128:(k + 1) * 128, :])
                X.append(xt)
            bias_t = cpool.tile([128, NUM_BLOCKS * KT], F32, name="bias_t")
            nc.scalar.dma_start(bias_t[:], bias_cols[:])
            warm = cpool.tile([128, 1], F32, name="warm")
            nc.gpsimd.memset(warm[:], 0.0)
            nc.scalar.activation(warm[:], warm[:], GELU)
            nc.scalar.activation(warm[:], warm[:], TANH)
            fc1b_t = cpool.tile([128, A], F32, name="fc1b_t")
            nc.gpsimd.dma_start(fc1b_t[:], fc1b[:])
            gbc_f = cpool.tile([128, A], F32, name="gbc_f")
            nc.gpsimd.dma_start(gbc_f[:], gbc[:])
            gbc_t = cpool.tile([128, A], BF16, name="gbc_t")
            with nc.allow_low_precision(reason="gamma rounds to bf16"):
                nc.vector.tensor_copy(gbc_t[:], gbc_f[:])
            # per-target fc weights are DMA'd mid-run (3 blocks ahead of use)
            fc1w_t, fc2w_t, fc2bias_t = {}, {}, {}

            anchors = {}   # a -> [tile per k]
            adiff = {}

            for i in range(NUM_BLOCKS):
                t_idx = target_of.get(i)
                a_idx = anchor_of.get(i)

                # prefetch the router weights for a target ~3 blocks out (on
                # the sync queue behind that block's weights — gpsimd must
                # stay free for routing math, scalar for gelus)
                if i + 3 in target_of:
                    tt = target_of[i + 3]
                    f1 = cpool.tile([128, KT * 128], BF16, name=f"fc1w_{tt}")
                    nc.sync.dma_start(
                        f1[:], fc1w[:, tt * KT * 128:(tt + 1) * KT * 128])
                    fc1w_t[tt] = f1
                    f2 = cpool.tile([128, A * C], BF16, name=f"fc2w_{tt}")
                    nc.sync.dma_start(
                        f2[:], fc2w[:, tt * A * C:(tt + 1) * A * C])
                    fc2w_t[tt] = f2
                    fb = cpool.tile([128, A * KT * BL], F32, name=f"fc2b_{tt}")
                    nc.sync.dma_start(
                        fb[:], fc2bias[:, tt * A * KT * BL:(tt + 1) * A * KT * BL])
                    fc2bias_t[tt] = fb

                # block weights: lhsT slice for (k, ct) at cols k*512 + ct*128
                if i == 0:
                    wsl = lambda k, ct: w0k[k][:, ct * 128:(ct + 1) * 128]
                else:
                    w_t = wpool.tile([128, KT * C], BF16, tag="w", name=f"w{i}")
                    for k in range(KT):
                        nc.sync.dma_start(w_t[:, k * C:(k + 1) * C],
                                          wd[i, k * 128:(k + 1) * 128, :])
                    wsl = (lambda wt: lambda k, ct:
                           wt[:, k * C + ct * 128:k * C + (ct + 1) * 128])(w_t)

                Xn = []
                for ct in range(KT):
                    if a_idx is not None:
                        xn = apool.tile([128, T], BF16, tag=f"a{a_idx}_{ct}",
                                        name=f"anc{a_idx}_{ct}")
                    else:
                        xn = xpool.tile([128, T], BF16, tag=f"x{ct}",
                                        name=f"xb{i}_{ct}")
                    Xn.append(xn)

                pooled = [None] * KT
                Xr = None
                if t_idx is not None:
                    # the routed update's tiles exist up front: the
                    # gate-independent base term (xr = Xn + gamma*a2) is
                    # emitted per-chunk right after each gelu, filling the
                    # DVE during the matmuls and leaving the softmax chain
                    # unqueued behind it
                    Xr = [xpool.tile([128, T], BF16, tag=f"x{k}",
                                     name=f"xr{t_idx}_{k}")
                          for k in range(KT)]
                if i in post_target:
                    # sample-major: each sample's matmuls run as soon as the
                    # DVE finishes that sample's router corrections, hiding
                    # the serial correction stream under PE work
                    PS = {}
                    for b in range(BL):
                        bsl = slice(b * HW, (b + 1) * HW)
                        po = b * 256
                        for ct in range(KT):
                            if b == 0:
                                PS[ct] = ppool.tile(
                                    [128, 1024], F32,
                                    tag="fc" if ct == 3 else "mm",
                                    bufs=1 if ct == 3 else 3,
                                    name=f"ps{i}_{ct}")
                            ps = PS[ct]
                            for k in range(KT):
                                nc.tensor.matmul(
                                    ps[:, po:po + HW],
                                    wsl(k, ct),
                                    X[k][:, bsl],
                                    start=(k == 0), stop=(k == KT - 1))
                            if b == BL - 1:
                                pv = ps.rearrange("p (n c) -> p n c",
                                                  n=BL)[:, :, 0:HW]
                                xv = Xn[ct].rearrange("p (b m) -> p b m", b=BL)
                                nc.scalar.activation(
                                    xv, pv, GELU,
                                    bias=bias_t[:, i * KT + ct:i * KT + ct + 1])
                else:
                    for ct in range(KT):
                        ps = ppool.tile([128, 1024], F32, tag="mm", bufs=3,
                                        name=f"ps{i}_{ct}")
                        for c in range(2):
                            for k in range(KT):
                                nc.tensor.matmul(
                                    ps[:, c * 512:c * 512 + CH],
                                    wsl(k, ct),
                                    X[k][:, c * CH:(c + 1) * CH],
                                    start=(k == 0), stop=(k == KT - 1))
                        bias_ap = bias_t[:, i * KT + ct:i * KT + ct + 1]
                        if t_idx is not None and ct == KT - 1:
                            # last ct of a target block: per-chunk gelu +
                            # per-chunk partial pool to shorten the fc1 tail
                            pl = rpool.tile([128, BL], BF16, tag=f"pool{ct}",
                                            name=f"pool{t_idx}_{ct}")
                            for c in range(2):
                                csl = slice(c * CH, (c + 1) * CH)
                                nc.scalar.activation(
                                    Xn[ct][:, csl],
                                    ps[:, c * 512:c * 512 + CH], GELU,
                                    bias=bias_ap)
                                with nc.allow_low_precision(
                                        reason="pooled rounds to bf16"):
                                    nc.vector.reduce_sum(
                                        pl[:, 2 * c:2 * c + 2],
                                        Xn[ct][:, csl]
                                        .rearrange("p (b m) -> p b m", b=2),
                                        axis=mybir.AxisListType.X)
                                nc.vector.scalar_tensor_tensor(
                                    Xr[ct][:, csl], anchors[2][ct][:, csl],
                                    gbc_t[:, t_idx:t_idx + 1], Xn[ct][:, csl],
                                    op0=mybir.AluOpType.mult,
                                    op1=mybir.AluOpType.add)
                            pooled[ct] = pl
                            continue
                        # one fat gelu across both PSUM banks
                        pv = ps.rearrange("p (n c) -> p n c", n=2)[:, :, 0:CH]
                        xv = Xn[ct].rearrange("p (n c) -> p n c", n=2)
                        nc.scalar.activation(xv, pv, GELU, bias=bias_ap)
                        if t_idx is not None:
                            # mean pool this ct right away (divisor folded
                            # into fc1w host-side); bf16 out for the router mm
                            pl = rpool.tile([128, BL], BF16, tag=f"pool{ct}",
                                            name=f"pool{t_idx}_{ct}")
                            with nc.allow_low_precision(
                                    reason="pooled rounds to bf16 on write"):
                                nc.vector.reduce_sum(
                                    pl[:],
                                    Xn[ct][:].rearrange("p (b m) -> p b m", b=BL),
                                    axis=mybir.AxisListType.X)
                            pooled[ct] = pl
                            nc.vector.scalar_tensor_tensor(
                                Xr[ct][:], anchors[2][ct][:],
                                gbc_t[:, t_idx:t_idx + 1], Xn[ct][:],
                                op0=mybir.AluOpType.mult,
                                op1=mybir.AluOpType.add)

                if a_idx is not None:
                    anchors[a_idx] = Xn
                    if a_idx == 2:
                        # precompute anchor differences (gates sum to gamma:
                        # routed = gamma*a2 + g0*(a0-a2) + g1*(a1-a2)) IN
                        # PLACE over a0/a1, whose raw values are dead now
                        for da in range(2):
                            adiff[da] = []
                            for k in range(KT):
                                dt_ = anchors[da][k]
                                nc.vector.tensor_sub(dt_[:], dt_[:],
                                                     anchors[2][k][:])
                                adiff[da].append(dt_)
                if t_idx is not None:
                    _routing(nc, rpool, ppool, t_idx, Xr, pooled,
                             adiff, fc1w_t, fc1b_t, fc2w_t,
                             fc2bias_t, gbc_f,
                             outT if i == NUM_BLOCKS - 1 else None)
                    Xn = Xr
                X = Xn

    nc.compile()
    return nc


def _routing(nc, rpool, ppool, t, Xr, pooled, adiff,
             fc1w_t, fc1b_t, fc2w_t, fc2bias_t, gbc_f, outT=None):
    """ChannelGating router: (precomputed) mean pool -> 2-layer MLP ->
    softmax over anchors -> per-sample gated anchor corrections applied
    in place over the (precomputed) base tiles Xr."""
    mul = mybir.AluOpType.mult
    add = mybir.AluOpType.add

    # fc1: h = gelu(pooled @ fc1_w + fc1_b)   [HID=128, BL]
    ps1 = ppool.tile([128, BL], F32, tag="fc", bufs=1, name=f"ps1_{t}")
    for k in range(KT):
        nc.tensor.matmul(ps1[:], fc1w_t[t][:, k * 128:(k + 1) * 128],
                         pooled[k][:], start=(k == 0), stop=(k == KT - 1))
    h = rpool.tile([128, BL], BF16, tag="h", name=f"h_{t}")
    nc.scalar.activation(h[:], ps1[:], GELU, bias=fc1b_t[:, t:t + 1])

    # fc2: logits [A*C, BL] as 12 col-tiles of one [128, 48] psum
    NJ = A * KT  # 12
    ps2 = ppool.tile([128, NJ * BL], F32, tag="fc", bufs=1, name=f"ps2_{t}")
    for j in range(NJ):
        nc.tensor.matmul(ps2[:, j * BL:(j + 1) * BL],
                         fc2w_t[t][:, j * 128:(j + 1) * 128],
                         h[:], start=True, stop=True)
    logits = rpool.tile([128, NJ * BL], F32, tag="logits", name=f"lg_{t}")
    nc.vector.tensor_add(logits[:], ps2[:], fc2bias_t[t][:])

    # softmax over a (cols = a*16 + k*4 + b), exp via tanh identity:
    # e^x = (1 + tanh(x/2)) / (1 - tanh(x/2)); logits are O(0.1) here so
    # the max-subtraction is skipped (tanh path is stable to |x|~17)
    th = rpool.tile([128, A * KB], F32, tag="th", name=f"th_{t}")
    nc.scalar.activation(th[:], logits[:], TANH, scale=0.5)
    den = rpool.tile([128, A * KB], F32, tag="den", name=f"den_{t}")
    nc.vector.tensor_scalar(den[:], th[:], -1.0, 1.0, op0=mul, op1=add)
    rec = rpool.tile([128, A * KB], F32, tag="rec", name=f"rec_{t}")
    nc.vector.reciprocal_approx_fast(rec[:], den[:])
    e = rpool.tile([128, A * KB], F32, tag="e", name=f"e_{t}")
    nc.vector.tensor_scalar(e[:], rec[:], 2.0, -1.0, op0=mul, op1=add)
    s = rpool.tile([128, KB], F32, tag="s", name=f"s_{t}")
    nc.vector.tensor_reduce(s[:], e[:].rearrange("p (a kb) -> p kb a", a=A),
                            axis=mybir.AxisListType.X, op=add)
    rinv = rpool.tile([128, KB], F32, tag="rinv", name=f"rinv_{t}")
    nc.vector.reciprocal_approx_fast(rinv[:], s[:])
    rg = rpool.tile([128, KB], F32, tag="rg", name=f"rg_{t}")
    nc.vector.tensor_scalar_mul(rg[:], rinv[:], gbc_f[:, t:t + 1])
    g = rpool.tile([128, 2 * KB], BF16, tag="g", name=f"g_{t}")
    for a in range(2):
        nc.vector.tensor_mul(g[:, a * KB:(a + 1) * KB],
                             e[:, a * KB:(a + 1) * KB], rg[:])

    # per-sample corrections, sample-major so the next block's per-sample
    # matmul passes (or the final output DMAs) start as early as possible:
    # xr[:, b] += g0[b]*(a0-a2)[:, b] + g1[b]*(a1-a2)[:, b]
    for b in range(BL):
        sl = slice(b * HW, (b + 1) * HW)
        for k in range(KT):
            xr = Xr[k]
            for a in range(2):
                col = a * KB + k * BL + b
                nc.vector.scalar_tensor_tensor(
                    xr[:, sl], adiff[a][k][:, sl],
                    g[:, col:col + 1], xr[:, sl], op0=mul, op1=add)
        if outT is not None and b % 2 == 1:
            c = b // 2
            csl = slice(c * CH, (c + 1) * CH)
            for k in range(KT):
                dq = nc.sync if k % 2 == 0 else nc.scalar
                dq.dma_start(outT[k * 128:(k + 1) * 128, csl],
                             Xr[k][:, csl])


def _prep_shared(block_w, block_b, fc1_w, fc1_b, fc2_w, fc2_b, gammas):
    """Host-side packing of the (replicated) weight tensors."""
    import ml_dtypes
    f = np.float32
    bf = ml_dtypes.bfloat16
    wd = np.ascontiguousarray(np.asarray(block_w, dtype=f).astype(bf))
    # bias column (i*KT+ct) = block_b[i, ct*128:(ct+1)*128]
    bias_cols = np.ascontiguousarray(
        np.asarray(block_b, dtype=f).reshape(NUM_BLOCKS * KT, 128).T, dtype=f)
    # fc1 with the mean-pool divisor folded in; col block (t*KT+k)
    fc1s = (np.asarray(fc1_w, dtype=f) / float(HW)).astype(f)   # [A, C, HID]
    fc1w_cat = np.concatenate(
        [fc1s[t][k * 128:(k + 1) * 128, :] for t in range(A) for k in range(KT)],
        axis=1)                                                 # [128, A*KT*128]
    fc1b_cols = np.ascontiguousarray(np.asarray(fc1_b, dtype=f).T)  # [128, A]
    fc2w_cat = np.concatenate([np.asarray(fc2_w[t], dtype=f) for t in range(A)],
                              axis=1)                           # [128, A*A*C]
    # fc2 bias expanded to the [128, (a,k,b)] logits layout, repeated per b
    fc2bias = np.concatenate(
        [np.repeat(np.asarray(fc2_b[t], dtype=f).reshape(A * KT, 128).T,
                   BL, axis=1) for t in range(A)], axis=1)      # [128, A*A*KT*BL]
    gbc = np.broadcast_to(np.asarray(gammas, dtype=f)[None, :], (128, A))
    return dict(wd=wd, bias_cols=bias_cols,
                fc1w=np.ascontiguousarray(fc1w_cat.astype(bf)),
                fc1b=fc1b_cols,
                fc2w=np.ascontiguousarray(fc2w_cat.astype(bf)),
                fc2bias=np.ascontiguousarray(fc2bias),
                gbc=np.ascontiguousarray(gbc))


def shard_x(x):
    """Full x [B,H,W,C] -> per-core transposed bf16 shards [C, T]."""
    import ml_dtypes
    shards = []
    for r in range(N_CORES):
        xs = np.asarray(x[r * BL:(r + 1) * BL], dtype=np.float32)  # [BL,H,W,C]
        shards.append(np.ascontiguousarray(
            xs.reshape(T, C).T.astype(ml_dtypes.bfloat16)))        # [C, T]
    return shards


def unshard_out(outs):
    """Per-core [C, T] results -> full [B,H,W,C]."""
    parts = [np.asarray(o, dtype=np.float32).T.reshape(BL, H, W, C)
             for o in outs]
    return np.ascontiguousarray(np.concatenate(parts, axis=0), dtype=np.float32)


def kernel(x, block_w, block_b, fc1_w, fc1_b, fc2_w, fc2_b, gammas):
    if "nc" not in _cached:
        _cached["nc"] = build_program()
    nc = _cached["nc"]

    shared = _prep_shared(block_w, block_b, fc1_w, fc1_b, fc2_w, fc2_b, gammas)
    xs = shard_x(x)
    in_maps = [dict(shared, xT=xs[r]) for r in range(N_CORES)]
    res = run_bass_kernel_spmd(nc, in_maps, list(range(N_CORES)))
    return unshard_out([res.results[r]["outT"] for r in range(N_CORES)])


# revision 21
# speedup vs baseline: 1.0213x; 1.0213x over previous
"""Trainium2 Bass kernel for DynamicResidualStageWrapper (18-block MLP stage
with channel-gated anchor routing), data-parallel over batch across 8 cores.

Contract: kernel(**inputs) takes FULL unsharded inputs (as numpy arrays, keyed
as in reference.setup_inputs()) and returns the FULL output [32,14,14,512].

Per-core layout: activations live transposed as [C=512, tokens=784] split into
4 partition-tiles [128, 784] in bf16; tokens are (sample b, position hw).
Block weights [cin, cout] are the natural lhsT for out[cout, tok] = W.T @ X.
The whole block path runs in bf16 (fp32 PSUM accumulate): rel-err ~4e-3 vs
the 2e-2 gate, and bf16 halves HBM traffic, halves LDWEIGHTS (FWL), and
doubles DVE throughput. Each (block, ct) uses ONE two-bank PSUM tile
[128,1024] (chunk01 at cols 0:392, chunk23 at 512:904) so gelu is a single
fat ACT instruction per ct — ACT is the closest co-bottleneck to the PE.
Post-target blocks (12, 15) are emitted chunk-major so the PE consumes the
router corrections at 2-sample granularity while the DVE produces them.
Softmax's exp uses e^x = (1+tanh(x/2))/(1-tanh(x/2)) so the scalar engine
never switches activation-table sets away from gelu/tanh.
"""

import numpy as np

import concourse.bacc as bacc
import concourse.mybir as mybir
import concourse.tile as tile
from concourse.bass_utils import run_bass_kernel_spmd

# ---- problem constants (hardcoded per spec) ----
NUM_BLOCKS = 18
ANCHOR_IDX = (1, 4, 9)
TARGET_IDX = (11, 14, 17)
C = 512
HID = 128
A = 3
B, H, W = 32, 14, 14
N_CORES = 8
BL = B // N_CORES          # 4 samples per core
HW = H * W                 # 196 positions per sample
T = BL * HW                # 784 tokens per core
KT = C // 128              # 4 channel tiles
CH = T // 2                # 392 tokens per chunk (2 samples)
KB = KT * BL               # 16 (k, b) gate columns per anchor

F32 = mybir.dt.float32
BF16 = mybir.dt.bfloat16
GELU = mybir.ActivationFunctionType.Gelu_apprx_tanh
TANH = mybir.ActivationFunctionType.Tanh
_cached = {}


def build_program():
    """Build the per-core Bass/Tile program (same program on all 8 cores)."""
    nc = bacc.Bacc(trn_type="TRN2", target_bir_lowering=False, debug=False)

    xT = nc.dram_tensor("xT", [C, T], BF16, kind="ExternalInput").ap()
    wd = nc.dram_tensor("wd", [NUM_BLOCKS, C, C], BF16, kind="ExternalInput").ap()
    bias_cols = nc.dram_tensor("bias_cols", [128, NUM_BLOCKS * KT], F32,
                               kind="ExternalInput").ap()
    fc1w = nc.dram_tensor("fc1w", [128, A * KT * 128], BF16, kind="ExternalInput").ap()
    fc1b = nc.dram_tensor("fc1b", [128, A], F32, kind="ExternalInput").ap()
    fc2w = nc.dram_tensor("fc2w", [128, A * A * C], BF16, kind="ExternalInput").ap()
    fc2bias = nc.dram_tensor("fc2bias", [128, A * A * KT * BL], F32,
                             kind="ExternalInput").ap()
    gbc = nc.dram_tensor("gbc", [128, A], F32, kind="ExternalInput").ap()
    outT = nc.dram_tensor("outT", [C, T], BF16, kind="ExternalOutput").ap()

    anchor_of = {b: i for i, b in enumerate(ANCHOR_IDX)}
    target_of = {b: i for i, b in enumerate(TARGET_IDX)}
    post_target = tuple(i + 1 for i in TARGET_IDX if i + 1 < NUM_BLOCKS)

    with tile.TileContext(nc) as tc:
        with (
            tc.tile_pool(name="const", bufs=1) as cpool,
            tc.tile_pool(name="wpool", bufs=6) as wpool,
            tc.tile_pool(name="xpool", bufs=2) as xpool,
            tc.tile_pool(name="apool", bufs=1) as apool,
            tc.tile_pool(name="rpool", bufs=2) as rpool,
            tc.tile_pool(name="ppool", bufs=3, space="PSUM") as ppool,
        ):
            # ---- startup DMAs: block-0 weights as four separate k-tiles so
            # the first matmul is gated by one 128KB DMA, not all four;
            # x tiles split across the scalar/gpsimd queues, router constants
            # behind them. ACT table warm-up (gelu+tanh) hides under the DMAs.
            w0k = []
            for k in range(KT):
                wk = cpool.tile([128, C], BF16, name=f"w0k{k}")
                nc.sync.dma_start(wk[:], wd[0, k * 128:(k + 1) * 128, :])
                w0k.append(wk)
            # all first-chunk halves of x land before any second half so
            # block 0's first matmul group is never input-starved
            X = [xpool.tile([128, T], BF16, tag=f"x{k}", name=f"xin{k}")
                 for k in range(KT)]
            for c in range(2):
                for k in range(KT):
                    csl = slice(c * CH, (c + 1) * CH)
                    eng = nc.scalar if k % 2 == 0 else nc.gpsimd
                    eng.dma_start(X[k][:, csl], xT[k * 128:(k + 1) * 128, csl])
            bias_t = cpool.tile([128, NUM_BLOCKS * KT], F32, name="bias_t")
            nc.scalar.dma_start(bias_t[:], bias_cols[:])
            warm = cpool.tile([128, 1], F32, name="warm")
            nc.gpsimd.memset(warm[:], 0.0)
            nc.scalar.activation(warm[:], warm[:], GELU)
            nc.scalar.activation(warm[:], warm[:], TANH)
            fc1b_t = cpool.tile([128, A], F32, name="fc1b_t")
            nc.gpsimd.dma_start(fc1b_t[:], fc1b[:])
            gbc_f = cpool.tile([128, A], F32, name="gbc_f")
            nc.gpsimd.dma_start(gbc_f[:], gbc[:])
            gbc_t = cpool.tile([128, A], BF16, name="gbc_t")
            with nc.allow_low_precision(reason="gamma rounds to bf16"):
                nc.vector.tensor_copy(gbc_t[:], gbc_f[:])
            # per-target fc weights are DMA'd mid-run (3 blocks ahead of use)
            fc1w_t, fc2w_t, fc2bias_t = {}, {}, {}

            anchors = {}   # a -> [tile per k]
            adiff = {}

            for i in range(NUM_BLOCKS):
                t_idx = target_of.get(i)
                a_idx = anchor_of.get(i)

                # prefetch the router weights for a target ~3 blocks out (on
                # the sync queue behind that block's weights — gpsimd must
                # stay free for routing math, scalar for gelus)
                if i + 3 in target_of:
                    tt = target_of[i + 3]
                    f1 = cpool.tile([128, KT * 128], BF16, name=f"fc1w_{tt}")
                    nc.sync.dma_start(
                        f1[:], fc1w[:, tt * KT * 128:(tt + 1) * KT * 128])
                    fc1w_t[tt] = f1
                    f2 = cpool.tile([128, A * C], BF16, name=f"fc2w_{tt}")
                    nc.sync.dma_start(
                        f2[:], fc2w[:, tt * A * C:(tt + 1) * A * C])
                    fc2w_t[tt] = f2
                    fb = cpool.tile([128, A * KT * BL], F32, name=f"fc2b_{tt}")
                    nc.sync.dma_start(
                        fb[:], fc2bias[:, tt * A * KT * BL:(tt + 1) * A * KT * BL])
                    fc2bias_t[tt] = fb

                # block weights: lhsT slice for (k, ct) at cols k*512 + ct*128
                if i == 0:
                    wsl = lambda k, ct: w0k[k][:, ct * 128:(ct + 1) * 128]
                else:
                    w_t = wpool.tile([128, KT * C], BF16, tag="w", name=f"w{i}")
                    for k in range(KT):
                        nc.sync.dma_start(w_t[:, k * C:(k + 1) * C],
                                          wd[i, k * 128:(k + 1) * 128, :])
                    wsl = (lambda wt: lambda k, ct:
                           wt[:, k * C + ct * 128:k * C + (ct + 1) * 128])(w_t)

                Xn = []
                for ct in range(KT):
                    if a_idx is not None:
                        xn = apool.tile([128, T], BF16, tag=f"a{a_idx}_{ct}",
                                        name=f"anc{a_idx}_{ct}")
                    else:
                        xn = xpool.tile([128, T], BF16, tag=f"x{ct}",
                                        name=f"xb{i}_{ct}")
                    Xn.append(xn)

                pooled = [None] * KT
                Xr = None
                if t_idx is not None:
                    # the routed update's tiles exist up front: the
                    # gate-independent base term (xr = Xn + gamma*a2) is
                    # emitted per-chunk right after each gelu, filling the
                    # DVE during the matmuls and leaving the softmax chain
                    # unqueued behind it
                    Xr = [xpool.tile([128, T], BF16, tag=f"x{k}",
                                     name=f"xr{t_idx}_{k}")
                          for k in range(KT)]
                if i in post_target:
                    # sample-major: each sample's matmuls run as soon as the
                    # DVE finishes that sample's router corrections, hiding
                    # the serial correction stream under PE work
                    PS = {}
                    for b in range(BL):
                        bsl = slice(b * HW, (b + 1) * HW)
                        po = b * 256
                        for ct in range(KT):
                            if b == 0:
                                PS[ct] = ppool.tile(
                                    [128, 1024], F32,
                                    tag="fc" if ct == 3 else "mm",
                                    bufs=1 if ct == 3 else 3,
                                    name=f"ps{i}_{ct}")
                            ps = PS[ct]
                            for k in range(KT):
                                nc.tensor.matmul(
                                    ps[:, po:po + HW],
                                    wsl(k, ct),
                                    X[k][:, bsl],
                                    start=(k == 0), stop=(k == KT - 1))
                            if b % 2 == 1:
                                # gelu each sample pair as it completes so
                                # the next block's chunk passes aren't gated
                                # on an end-of-block gelu pile-up
                                c = b // 2
                                pv = (ps.rearrange("p (n c) -> p n c", n=BL)
                                      [:, 2 * c:2 * c + 2, 0:HW])
                                xv = (Xn[ct][:, c * CH:(c + 1) * CH]
                                      .rearrange("p (b m) -> p b m", b=2))
                                nc.scalar.activation(
                                    xv, pv, GELU,
                                    bias=bias_t[:, i * KT + ct:i * KT + ct + 1])
                else:
                    for ct in range(KT):
                        ps = ppool.tile([128, 1024], F32, tag="mm", bufs=3,
                                        name=f"ps{i}_{ct}")
                        for c in range(2):
                            for k in range(KT):
                                nc.tensor.matmul(
                                    ps[:, c * 512:c * 512 + CH],
                                    wsl(k, ct),
                                    X[k][:, c * CH:(c + 1) * CH],
                                    start=(k == 0), stop=(k == KT - 1))
                        bias_ap = bias_t[:, i * KT + ct:i * KT + ct + 1]
                        if t_idx is not None and ct == KT - 1:
                            # last ct of a target block: per-chunk gelu +
                            # per-chunk partial pool to shorten the fc1 tail
                            pl = rpool.tile([128, BL], BF16, tag=f"pool{ct}",
                                            name=f"pool{t_idx}_{ct}")
                            for c in range(2):
                                csl = slice(c * CH, (c + 1) * CH)
                                nc.scalar.activation(
                                    Xn[ct][:, csl],
                                    ps[:, c * 512:c * 512 + CH], GELU,
                                    bias=bias_ap)
                                with nc.allow_low_precision(
                                        reason="pooled rounds to bf16"):
                                    nc.vector.reduce_sum(
                                        pl[:, 2 * c:2 * c + 2],
                                        Xn[ct][:, csl]
                                        .rearrange("p (b m) -> p b m", b=2),
                                        axis=mybir.AxisListType.X)
                                nc.vector.scalar_tensor_tensor(
                                    Xr[ct][:, csl], anchors[2][ct][:, csl],
                                    gbc_t[:, t_idx:t_idx + 1], Xn[ct][:, csl],
                                    op0=mybir.AluOpType.mult,
                                    op1=mybir.AluOpType.add)
                            pooled[ct] = pl
                            continue
                        # one fat gelu across both PSUM banks
                        pv = ps.rearrange("p (n c) -> p n c", n=2)[:, :, 0:CH]
                        xv = Xn[ct].rearrange("p (n c) -> p n c", n=2)
                        nc.scalar.activation(xv, pv, GELU, bias=bias_ap)
                        if t_idx is not None:
                            # mean pool this ct right away (divisor folded
                            # into fc1w host-side); bf16 out for the router mm
                            pl = rpool.tile([128, BL], BF16, tag=f"pool{ct}",
                                            name=f"pool{t_idx}_{ct}")
                            with nc.allow_low_precision(
                                    reason="pooled rounds to bf16 on write"):
                                nc.vector.reduce_sum(
                                    pl[:],
                                    Xn[ct][:].rearrange("p (b m) -> p b m", b=BL),
                                    axis=mybir.AxisListType.X)
                            pooled[ct] = pl
                            nc.vector.scalar_tensor_tensor(
                                Xr[ct][:], anchors[2][ct][:],
                                gbc_t[:, t_idx:t_idx + 1], Xn[ct][:],
                                op0=mybir.AluOpType.mult,
                                op1=mybir.AluOpType.add)

                if a_idx is not None:
                    anchors[a_idx] = Xn
                    if a_idx == 2:
                        # precompute anchor differences (gates sum to gamma:
                        # routed = gamma*a2 + g0*(a0-a2) + g1*(a1-a2)) IN
                        # PLACE over a0/a1, whose raw values are dead now
                        for da in range(2):
                            adiff[da] = []
                            for k in range(KT):
                                dt_ = anchors[da][k]
                                nc.vector.tensor_sub(dt_[:], dt_[:],
                                                     anchors[2][k][:])
                                adiff[da].append(dt_)
                if t_idx is not None:
                    _routing(nc, rpool, ppool, t_idx, Xr, pooled,
                             adiff, fc1w_t, fc1b_t, fc2w_t,
                             fc2bias_t, gbc_f,
                             outT if i == NUM_BLOCKS - 1 else None)
                    Xn = Xr
                X = Xn

    nc.compile()
    return nc


def _routing(nc, rpool, ppool, t, Xr, pooled, adiff,
             fc1w_t, fc1b_t, fc2w_t, fc2bias_t, gbc_f, outT=None):
    """ChannelGating router: (precomputed) mean pool -> 2-layer MLP ->
    softmax over anchors -> per-sample gated anchor corrections applied
    in place over the (precomputed) base tiles Xr."""
    mul = mybir.AluOpType.mult
    add = mybir.AluOpType.add

    # fc1: h = gelu(pooled @ fc1_w + fc1_b)   [HID=128, BL]
    ps1 = ppool.tile([128, BL], F32, tag="fc", bufs=1, name=f"ps1_{t}")
    for k in range(KT):
        nc.tensor.matmul(ps1[:], fc1w_t[t][:, k * 128:(k + 1) * 128],
                         pooled[k][:], start=(k == 0), stop=(k == KT - 1))
    h = rpool.tile([128, BL], BF16, tag="h", name=f"h_{t}")
    nc.scalar.activation(h[:], ps1[:], GELU, bias=fc1b_t[:, t:t + 1])

    # fc2: logits [A*C, BL] as 12 col-tiles of one [128, 48] psum
    NJ = A * KT  # 12
    ps2 = ppool.tile([128, NJ * BL], F32, tag="fc", bufs=1, name=f"ps2_{t}")
    for j in range(NJ):
        nc.tensor.matmul(ps2[:, j * BL:(j + 1) * BL],
                         fc2w_t[t][:, j * 128:(j + 1) * 128],
                         h[:], start=True, stop=True)
    logits = rpool.tile([128, NJ * BL], F32, tag="logits", name=f"lg_{t}")
    nc.vector.tensor_add(logits[:], ps2[:], fc2bias_t[t][:])

    # softmax over a (cols = a*16 + k*4 + b), exp via tanh identity:
    # e^x = (1 + tanh(x/2)) / (1 - tanh(x/2)); logits are O(0.1) here so
    # the max-subtraction is skipped (tanh path is stable to |x|~17)
    th = rpool.tile([128, A * KB], F32, tag="th", name=f"th_{t}")
    nc.scalar.activation(th[:], logits[:], TANH, scale=0.5)
    den = rpool.tile([128, A * KB], F32, tag="den", name=f"den_{t}")
    nc.vector.tensor_scalar(den[:], th[:], -1.0, 1.0, op0=mul, op1=add)
    rec = rpool.tile([128, A * KB], F32, tag="rec", name=f"rec_{t}")
    nc.vector.reciprocal_approx_fast(rec[:], den[:])
    e = rpool.tile([128, A * KB], F32, tag="e", name=f"e_{t}")
    nc.vector.tensor_scalar(e[:], rec[:], 2.0, -1.0, op0=mul, op1=add)
    s = rpool.tile([128, KB], F32, tag="s", name=f"s_{t}")
    nc.vector.tensor_reduce(s[:], e[:].rearrange("p (a kb) -> p kb a", a=A),
                            axis=mybir.AxisListType.X, op=add)
    rinv = rpool.tile([128, KB], F32, tag="rinv", name=f"rinv_{t}")
    nc.vector.reciprocal_approx_fast(rinv[:], s[:])
    rg = rpool.tile([128, KB], F32, tag="rg", name=f"rg_{t}")
    nc.vector.tensor_scalar_mul(rg[:], rinv[:], gbc_f[:, t:t + 1])
    g = rpool.tile([128, 2 * KB], BF16, tag="g", name=f"g_{t}")
    for a in range(2):
        nc.vector.tensor_mul(g[:, a * KB:(a + 1) * KB],
                             e[:, a * KB:(a + 1) * KB], rg[:])

    # per-sample corrections, sample-major so the next block's per-sample
    # matmul passes (or the final output DMAs) start as early as possible:
    # xr[:, b] += g0[b]*(a0-a2)[:, b] + g1[b]*(a1-a2)[:, b]
    for b in range(BL):
        sl = slice(b * HW, (b + 1) * HW)
        for k in range(KT):
            xr = Xr[k]
            for a in range(2):
                col = a * KB + k * BL + b
                nc.vector.scalar_tensor_tensor(
                    xr[:, sl], adiff[a][k][:, sl],
                    g[:, col:col + 1], xr[:, sl], op0=mul, op1=add)
        if outT is not None and b % 2 == 1:
            c = b // 2
            csl = slice(c * CH, (c + 1) * CH)
            for k in range(KT):
                dq = nc.sync if k % 2 == 0 else nc.scalar
                dq.dma_start(outT[k * 128:(k + 1) * 128, csl],
                             Xr[k][:, csl])


def _prep_shared(block_w, block_b, fc1_w, fc1_b, fc2_w, fc2_b, gammas):
    """Host-side packing of the (replicated) weight tensors."""
    import ml_dtypes
    f = np.float32
    bf = ml_dtypes.bfloat16
    wd = np.ascontiguousarray(np.asarray(block_w, dtype=f).astype(bf))
    # bias column (i*KT+ct) = block_b[i, ct*128:(ct+1)*128]
    bias_cols = np.ascontiguousarray(
        np.asarray(block_b, dtype=f).reshape(NUM_BLOCKS * KT, 128).T, dtype=f)
    # fc1 with the mean-pool divisor folded in; col block (t*KT+k)
    fc1s = (np.asarray(fc1_w, dtype=f) / float(HW)).astype(f)   # [A, C, HID]
    fc1w_cat = np.concatenate(
        [fc1s[t][k * 128:(k + 1) * 128, :] for t in range(A) for k in range(KT)],
        axis=1)                                                 # [128, A*KT*128]
    fc1b_cols = np.ascontiguousarray(np.asarray(fc1_b, dtype=f).T)  # [128, A]
    fc2w_cat = np.concatenate([np.asarray(fc2_w[t], dtype=f) for t in range(A)],
                              axis=1)                           # [128, A*A*C]
    # fc2 bias expanded to the [128, (a,k,b)] logits layout, repeated per b
    fc2bias = np.concatenate(
        [np.repeat(np.asarray(fc2_b[t], dtype=f).reshape(A * KT, 128).T,
                   BL, axis=1) for t in range(A)], axis=1)      # [128, A*A*KT*BL]
    gbc = np.broadcast_to(np.asarray(gammas, dtype=f)[None, :], (128, A))
    return dict(wd=wd, bias_cols=bias_cols,
                fc1w=np.ascontiguousarray(fc1w_cat.astype(bf)),
                fc1b=fc1b_cols,
                fc2w=np.ascontiguousarray(fc2w_cat.astype(bf)),
                fc2bias=np.ascontiguousarray(fc2bias),
                gbc=np.ascontiguousarray(gbc))


def shard_x(x):
    """Full x [B,H,W,C] -> per-core transposed bf16 shards [C, T]."""
    import ml_dtypes
    shards = []
    for r in range(N_CORES):
        xs = np.asarray(x[r * BL:(r + 1) * BL], dtype=np.float32)  # [BL,H,W,C]
        shards.append(np.ascontiguousarray(
            xs.reshape(T, C).T.astype(ml_dtypes.bfloat16)))        # [C, T]
    return shards


def unshard_out(outs):
    """Per-core [C, T] results -> full [B,H,W,C]."""
    parts = [np.asarray(o, dtype=np.float32).T.reshape(BL, H, W, C)
             for o in outs]
    return np.ascontiguousarray(np.concatenate(parts, axis=0), dtype=np.float32)


def kernel(x, block_w, block_b, fc1_w, fc1_b, fc2_w, fc2_b, gammas):
    if "nc" not in _cached:
        _cached["nc"] = build_program()
    nc = _cached["nc"]

    shared = _prep_shared(block_w, block_b, fc1_w, fc1_b, fc2_w, fc2_b, gammas)
    xs = shard_x(x)
    in_maps = [dict(shared, xT=xs[r]) for r in range(N_CORES)]
    res = run_bass_kernel_spmd(nc, in_maps, list(range(N_CORES)))
    return unshard_out([res.results[r]["outT"] for r in range(N_CORES)])
